# revision 1
# baseline (speedup 1.0000x reference)
"""Trainium2 Bass kernel for AccumulativeGainLoss.

Data-parallel over B across 8 NeuronCores (2 batch elements j=0,1 per core).
Measured ~60 us HW exec per core (incl ~15 us fixed Tile pre/postamble);
relative error vs the fp32 jax reference ~1.4e-4 (bf16 streaming).

Math restructuring (validated on host to ~2.5e-6 in f32 / ~1.6e-4 in bf16):
for each batch element, with F = preds[b] [N,K], Y = y_ts[b] as [N, T*D]:
    H   = [F|1]^T [F|1]                 (Gram + column sums, PE, PSUM-accum)
    inv = (F^T F)^{-1}                  (Newton-Schulz, 3 iters, X0=(K/tr)I)
    M   = F^T Y, sumy = 1^T Y, sy2 = 1^T (Y*Y)   (one fused PE pass)
    q   = colsum(M * (inv M))           (= diag(M^T inv M))
    ss_res = sy2 - q                    (beta^T FtF beta ~= beta^T M, err 1e-12)
    ss_tot = sy2 - sumy^2/N + EPS
    r2  = 1 - ss_res/ss_tot ;  wsum_b = sum_td w[t,d] * r2[t,d]
    cov = FtF - s s^T / N ; c = 1/diag(cov) ; quad_b = c^T (cov*cov) c
loss = mean_b( -wsum_b/T ) + 0.1 * mean_b( quad_b - K )

Implementation notes (hard-won on real TRN2):
- Host casts preds/y_ts to bf16 and lays them out partition-major
  ([p, chunk, t*D+d] images) so every DMA descriptor is a multi-KB
  per-partition contiguous run.  Output accuracy stays ~1.4e-4 because
  the r2 ratio is largely invariant to independent quantization of Y.
- Y streams in 16 x ~0.4 MB blocks chained depth-3 (each trigger waits
  the completion three links back): in-order arrivals every ~1.5-2 us at
  high aggregate bandwidth.  Unchained, the SDMA engines round-robin all
  queues and every block lands together at the end; fully serial chains
  pay the ~2 us completion receipt per block.
- One matmul per 128-row chunk with rhs spanning [Y | Y^2] via a
  two-level strided AP (free size 512 = one PSUM bank of fp32).
- Y^2 on ScalarE/VectorE alternating blocks; neither engine's serial
  backlog then trails the DMA stream.
- The Newton-Schulz + correlation-penalty chains are tiny matmul <-> DVE
  ping-pongs; emitted inline they head-of-line-block the PE FIFO, so
  their PE steps are interleaved into the streaming chunk loop (one step
  per 4 chunks) where their DVE inputs are long since ready.
- A junk-matmul warmup burst occupies the PE during the ~7 us Tile
  preamble + F load so the HAM clock-gate reaches 2.4 GHz before the
  real stream begins.
- TRN2 instruction encodings hold a single sync wait; bacc's
  generate_event_semaphores() splits multi-wait instructions (use Bacc +
  nc.compile(), not raw Bass, or walrus dies with "Too many sync wait
  commands").
- The scalar row epilogue lives on partition 32 (where the fused GS
  matmul leaves sumy/sy2); engines cannot move data across partitions.
"""

import ml_dtypes
import numpy as np

import concourse.bacc as bacc
import concourse.bass as bass
import concourse.mybir as mybir
import concourse.tile as tile
from concourse.bass_utils import run_bass_kernel_spmd
from concourse.tile_rust import add_dep_helper

F32 = mybir.dt.float32
BF16 = mybir.dt.bfloat16
ALU = mybir.AluOpType
AX = mybir.AxisListType

B, T, N, K, D = 16, 32, 6000, 32, 8
NCORES = 8
JB = B // NCORES          # batch elements per core
NCH = 47                  # ceil(6000/128) chunks of 128 rows
NPAD = NCH * 128          # 6016
TD = T * D                # 256
FW = 34                   # per-chunk F block: 32 coeffs + ones col + pad
FROW = NCH * FW           # 1598
YROW = NCH * TD           # 12032
BLOCKS_J = ((6, 6, 6, 6, 6, 6, 6, 5),
            (6, 6, 6, 6, 6, 6, 6, 5))
NS_ITERS = 3
EPS = 1e-8
DECAY = 0.9
PEN = 0.1

_CACHE = {}


def _build_program():
    nc = bacc.Bacc("TRN2", target_bir_lowering=False, debug=False)
    y_d = nc.declare_dram_parameter("y", [JB, 128, YROW], BF16, isOutput=False)
    f_d = nc.declare_dram_parameter("f", [JB, 128, FROW], BF16, isOutput=False)
    c_d = nc.declare_dram_parameter("c32", [32, 96], F32, isOutput=False)
    w_d = nc.declare_dram_parameter("w2", [1, TD], F32, isOutput=False)
    o_d = nc.declare_dram_parameter("out", [1, 2], F32, isOutput=True)

    with tile.TileContext(nc) as tc:
        with (
            tc.tile_pool(name="cpool", bufs=1) as cpool,
            tc.tile_pool(name="fpool", bufs=1) as fpool,
            tc.tile_pool(name="ypool", bufs=8) as ypool,
            tc.tile_pool(name="nsb", bufs=2) as nsb,
            tc.tile_pool(name="esb", bufs=2) as esb,
            # PSUM is 8 banks; every tag below occupies one bank.
            tc.tile_pool(name="ps", bufs=1, space="PSUM") as ps,
        ):
            # ---- PE warmup: junk matmuls fill the otherwise idle start
            # window so the HAM clock-gate reaches 2.4 GHz before the real
            # matmuls arrive (~3.4 us of sustained activity required).
            wtile = cpool.tile([128, 512], BF16)
            nc.gpsimd.memset(wtile, 0.01)
            wps = ps.tile([128, 512], F32, tag="wrm")
            for _ in range(14):
                nc.tensor.matmul(wps, wtile[:, 0:128], wtile,
                                 start=True, stop=True)

            # ---- DMAs: F first (needed by every matmul), then the Y
            # stream; triggers alternate between the two HWDGE issuing
            # engines (SP / ACT sequencer) so trigger issue is not serial
            # on one queue.
            ftile = fpool.tile([128, JB * FROW], BF16)
            fdmas = []
            for j in range(JB):
                fdmas.append(nc.sync.dma_start(
                    out=ftile[:, j * FROW:(j + 1) * FROW],
                    in_=f_d[j, :, :],
                ))
            fdma = fdmas[0]

            def fch(j, c):  # chunk-c F block [128, 33] (coeffs + ones)
                return ftile[:, j * FROW + c * FW: j * FROW + c * FW + 33]

            # ycomb tiles: [Y | Ysq] halves, one tile per (j, block).
            # The transfers are chained depth-2 (each trigger waits for the
            # completion two links back): at most two Y streams in flight,
            # so blocks arrive in order every ~2 us at full aggregate HBM
            # bandwidth instead of all 16 landing together at the end
            # (SDMA engines round-robin between all queues that have work).
            ycombs = {}
            ydmas = []
            dma_engines = [nc.sync, nc.scalar]
            for j in range(JB):
                c0 = 0
                for bi, blk in enumerate(BLOCKS_J[j]):
                    yc = ypool.tile([128, blk * 512], BF16, tag=f"yc{j}",
                                    bufs=len(BLOCKS_J[j]))
                    eng = dma_engines[len(ydmas) % 2]
                    dma = eng.dma_start(
                        out=yc[:, 0:blk * TD],
                        in_=y_d[j, :, c0 * TD:(c0 + blk) * TD],
                    )
                    k = len(ydmas)
                    if k < 2:
                        add_dep_helper(dma.ins, fdma.ins, sync=True,
                                       reason="F streams solo first")
                    elif k < 4:
                        add_dep_helper(dma.ins, ydmas[k - 2].ins, sync=True,
                                       reason="depth-2 ramp")
                    else:
                        add_dep_helper(dma.ins, ydmas[k - 3].ins, sync=True,
                                       reason="depth-3 Y stream chain")
                    ydmas.append(dma)
                    ycombs[(j, bi)] = yc
                    c0 += blk

            consts = cpool.tile([32, 96], F32)
            nc.gpsimd.dma_start(out=consts, in_=c_d[:, :])
            eye = consts[:, 0:32]
            twoI = consts[:, 32:64]
            ones2d = consts[:, 64:96]
            ones32 = consts[:, 64:65]

            w2sb = cpool.tile([33, TD], F32)
            nc.gpsimd.dma_start(out=w2sb[32:33, :], in_=w_d[:, :])
            sumw = cpool.tile([33, 1], F32)
            nc.vector.reduce_sum(sumw[32:33, :], w2sb[32:33, :], axis=AX.X)

            # ---- Newton-Schulz inverse of FtF + correlation penalty, per j.
            # The Gram matrices are computed up front (dense PE work), but
            # the serial NS/corr chains (tiny matmul <-> DVE ping-pong)
            # would head-of-line-block the PE FIFO if emitted as one run.
            # Each PE step is wrapped in a closure and interleaved into the
            # streaming chunk loop below, so every step's DVE inputs are
            # long finished before the PE reaches its matmul.
            inv_sb = [None, None]
            quad_sb = [None, None]
            Hsb_j = [None, None]

            def emit_H(j):
                Hps = ps.tile([33, 33], F32, tag=f"H{j}")
                for c in range(NCH):
                    nc.tensor.matmul(
                        Hps, fch(j, c), fch(j, c),
                        start=(c == 0), stop=(c == NCH - 1),
                    )
                Hsb = nsb.tile([33, 33], F32, tag="Hsb")
                nc.vector.tensor_copy(Hsb, Hps)
                Hsb_j[j] = Hsb

            def make_steps(j):
                state = {}

                def s_trace():
                    Hsb = Hsb_j[j]
                    A = state["A"] = Hsb[0:32, 0:32]
                    state["s_row"] = Hsb[32:33, 0:32]
                    dm = nsb.tile([32, 32], F32, tag="dm")
                    nc.vector.tensor_mul(dm, A, eye)
                    dg = nsb.tile([32, 1], F32, tag="dg")
                    nc.vector.reduce_sum(dg, dm, axis=AX.X)
                    trp = ps.tile([32, 32], F32, tag="tns", bufs=2)
                    nc.tensor.matmul(trp[:, 0:1], ones2d, dg,
                                     start=True, stop=True)
                    rtr = nsb.tile([32, 1], F32, tag="rtr")
                    nc.vector.reciprocal(rtr, trp[:, 0:1])
                    c0v = nsb.tile([32, 1], F32, tag="c0v")
                    nc.vector.tensor_scalar_mul(c0v, rtr, float(K))
                    X = nsb.tile([32, 32], F32, tag="Xns", bufs=2 * NS_ITERS + 4)
                    nc.vector.tensor_scalar(X, eye, c0v, None, ALU.mult)
                    state["X"] = X
                steps = [s_trace]

                def ns_a():
                    t1 = ps.tile([32, 32], F32, tag="tns", bufs=2)
                    nc.tensor.matmul(t1, state["A"], state["X"],
                                     start=True, stop=True)
                    z = nsb.tile([32, 32], F32, tag="Zns",
                                 bufs=2 * NS_ITERS + 2)
                    nc.vector.tensor_sub(z, twoI, t1)
                    state["z"] = z

                def ns_b():
                    x2 = ps.tile([32, 32], F32, tag="tns", bufs=2)
                    nc.tensor.matmul(x2, state["X"], state["z"],
                                     start=True, stop=True)
                    Xn = nsb.tile([32, 32], F32, tag="Xns",
                                  bufs=2 * NS_ITERS + 4)
                    nc.vector.tensor_copy(Xn, x2)
                    state["X"] = Xn
                for _ in range(NS_ITERS):
                    steps += [ns_a, ns_b]

                def c_outer():
                    inv_sb[j] = state["X"]
                    outp = ps.tile([32, 32], F32, tag="tns", bufs=2)
                    nc.tensor.matmul(outp, state["s_row"], state["s_row"],
                                     start=True, stop=True)
                    covn = nsb.tile([32, 32], F32, tag="covn")
                    nc.vector.tensor_scalar_mul(covn, outp, 1.0 / N)
                    cov = nsb.tile([32, 32], F32, tag="cov")
                    nc.vector.tensor_sub(cov, state["A"], covn)
                    dm2 = nsb.tile([32, 32], F32, tag="dm2")
                    nc.vector.tensor_mul(dm2, cov, eye)
                    dg2 = nsb.tile([32, 1], F32, tag="dg2")
                    nc.vector.reduce_sum(dg2, dm2, axis=AX.X)
                    cv = nsb.tile([32, 1], F32, tag="cv")
                    nc.vector.reciprocal(cv, dg2)
                    A2 = nsb.tile([32, 32], F32, tag="A2")
                    nc.vector.tensor_mul(A2, cov, cov)
                    state["cv"] = cv
                    state["A2"] = A2

                def c_u():
                    ups = ps.tile([32, 32], F32, tag="tns", bufs=2)
                    nc.tensor.matmul(ups[:, 0:1], state["A2"], state["cv"],
                                     start=True, stop=True)
                    usb = nsb.tile([32, 1], F32, tag="usb")
                    nc.vector.tensor_copy(usb, ups[:, 0:1])
                    state["usb"] = usb

                def c_q():
                    qd = ps.tile([33, 32], F32, tag="tns", bufs=2)
                    nc.tensor.matmul(qd[32:33, 0:1], state["usb"], state["cv"],
                                     start=True, stop=True)
                    qsb = nsb.tile([33, 1], F32, tag="qsb")
                    nc.vector.tensor_copy(qsb[32:33, :], qd[32:33, 0:1])
                    quad_sb[j] = qsb
                steps += [c_outer, c_u, c_q]
                return steps

            emit_H(0)
            emit_H(1)
            pending = {0: make_steps(0), 1: make_steps(1)}

            # results staging: [wsum0, wsum1, quad0, quad1] (on partition 32,
            # where the GS row outputs live)
            wsout = cpool.tile([33, 4], F32)

            # ---- stream: square each block (alternating ScalarE / DVE),
            # then one matmul per chunk with rhs spanning [Y | Ysq]:
            #   GS[0:32, 0:256]   = F^T Y   (M)
            #   GS[32,   0:256]   = 1^T Y   (sumy)
            #   GS[32,   256:512] = 1^T Y^2 (sy2)
            for j in range(JB):
                GS = ps.tile([33, 512], F32, tag=f"GS{j}")
                steps = pending.pop(j)
                c0 = 0
                for bi, blk in enumerate(BLOCKS_J[j]):
                    yc = ycombs[(j, bi)]
                    # squares alternate ScalarE / VectorE so neither
                    # engine's serial backlog trails the DMA stream
                    if (j * len(BLOCKS_J[0]) + bi) % 2 == 0:
                        nc.scalar.square(
                            yc[:, blk * TD:2 * blk * TD], yc[:, 0:blk * TD]
                        )
                    else:
                        nc.vector.tensor_mul(
                            yc[:, blk * TD:2 * blk * TD],
                            yc[:, 0:blk * TD], yc[:, 0:blk * TD]
                        )
                    rhs2 = yc[:, :].rearrange("p (two cd) -> p two cd", two=2)
                    for lc in range(blk):
                        c = c0 + lc
                        nc.tensor.matmul(
                            GS, fch(j, c),
                            rhs2[:, :, lc * TD:(lc + 1) * TD],
                            start=(c == 0), stop=(c == NCH - 1),
                        )
                        if c % 3 == 2 and steps:
                            steps.pop(0)()
                    c0 += blk
                while steps:
                    steps.pop(0)()

                # ---- per-j epilogue
                Gsb = esb.tile([33, 512], F32, tag="Gsb")
                nc.vector.tensor_copy(Gsb, GS)
                M = Gsb[0:32, 0:TD]
                sumy = Gsb[32:33, 0:TD]
                sy2row = Gsb[32:33, TD:2 * TD]

                Pps = ps.tile([32, TD], F32, tag="tPq")
                nc.tensor.matmul(Pps, inv_sb[j], M, start=True, stop=True)
                # ss_tot chain runs on DVE while PE computes P = inv M
                sumy2 = esb.tile([33, TD], F32, tag="sumy2")
                nc.vector.tensor_mul(sumy2[32:33, :], sumy, sumy)
                sstot_a = esb.tile([33, TD], F32, tag="sstot_a")
                nc.vector.tensor_scalar(
                    sstot_a[32:33, :], sumy2[32:33, :], -1.0 / N, EPS,
                    ALU.mult, ALU.add
                )
                sstot = esb.tile([33, TD], F32, tag="sstot")
                nc.vector.tensor_add(sstot[32:33, :], sstot_a[32:33, :], sy2row)
                rec = esb.tile([33, TD], F32, tag="rec")
                nc.vector.reciprocal(rec[32:33, :], sstot[32:33, :])
                # wsum = sum(w*r2) = sum(w) - sum(w*rec*sy2) + sum(w*rec*q);
                # everything except the q term hides under the P/q matmuls
                wrec = esb.tile([33, TD], F32, tag="wrec")
                nc.vector.tensor_mul(wrec[32:33, :], rec[32:33, :],
                                     w2sb[32:33, :])
                tA = esb.tile([33, TD], F32, tag="tA")
                accA = esb.tile([33, 1], F32, tag="accA")
                nc.vector.scalar_tensor_tensor(
                    tA[32:33, :], sy2row, 1.0, wrec[32:33, :],
                    ALU.mult, ALU.mult, accum_out=accA[32:33, :])
                W = esb.tile([32, TD], F32, tag="W")
                nc.vector.tensor_mul(W, M, Pps)
                qps = ps.tile([33, TD], F32, tag="tPq")
                nc.tensor.matmul(qps[32:33, :], ones32, W, start=True, stop=True)
                tB = esb.tile([33, TD], F32, tag="tB")
                accB = esb.tile([33, 1], F32, tag="accB")
                nc.vector.scalar_tensor_tensor(
                    tB[32:33, :], qps[32:33, :], 1.0, wrec[32:33, :],
                    ALU.mult, ALU.mult, accum_out=accB[32:33, :])
                d1 = esb.tile([33, 1], F32, tag="d1")
                nc.vector.tensor_sub(d1[32:33, :], sumw[32:33, :],
                                     accA[32:33, :])
                nc.vector.tensor_add(wsout[32:33, j:j + 1], d1[32:33, :],
                                     accB[32:33, :])
                nc.vector.tensor_copy(wsout[32:33, 2 + j:3 + j],
                                      quad_sb[j][32:33, :])

            outsb = cpool.tile([33, 2], F32)
            nc.vector.tensor_add(outsb[32:33, 0:1], wsout[32:33, 0:1],
                                 wsout[32:33, 1:2])
            nc.vector.tensor_add(outsb[32:33, 1:2], wsout[32:33, 2:3],
                                 wsout[32:33, 3:4])
            nc.sync.dma_start(out=o_d[:, :], in_=outsb[32:33, :])

    nc.compile()
    return nc


def _prepare_in_maps(preds, y_ts, importance):
    preds = np.ascontiguousarray(preds, dtype=np.float32)
    y_ts = np.ascontiguousarray(y_ts, dtype=np.float32)
    importance = np.ascontiguousarray(importance, dtype=np.float32)

    bf16 = ml_dtypes.bfloat16

    # Y image: yimg[b, p, c*TD + t*D + d] = y_ts[b, t, c*128+p, d]
    ypad = np.zeros((B, T, NPAD, D), dtype=bf16)
    ypad[:, :, :N, :] = y_ts.astype(bf16)
    yimg = np.ascontiguousarray(
        ypad.reshape(B, T, NCH, 128, D).transpose(0, 3, 2, 1, 4)
    ).reshape(B, 128, YROW)

    # F image: fimg[b, p, c*FW + k] = preds[b, c*128+p, k]; col 32 = valid-mask
    fpad = np.zeros((B, NPAD, FW), dtype=bf16)
    fpad[:, :N, :K] = preds.astype(bf16)
    fpad[:, :N, K] = 1.0
    fimg = np.ascontiguousarray(
        fpad.reshape(B, NCH, 128, FW).transpose(0, 2, 1, 3)
    ).reshape(B, 128, FROW)

    c32 = np.zeros((32, 96), dtype=np.float32)
    c32[:, 0:32] = np.eye(32, dtype=np.float32)
    c32[:, 32:64] = 2.0 * np.eye(32, dtype=np.float32)
    c32[:, 64:96] = 1.0

    decay = DECAY ** np.arange(T, dtype=np.float32)
    w2 = (decay[:, None] * importance[None, :].astype(np.float32)).reshape(1, TD)
    w2 = np.ascontiguousarray(w2, dtype=np.float32)

    in_maps = []
    for i in range(NCORES):
        in_maps.append({
            "y": np.ascontiguousarray(yimg[i * JB:(i + 1) * JB]),
            "f": np.ascontiguousarray(fimg[i * JB:(i + 1) * JB]),
            "c32": c32,
            "w2": w2,
        })
    return in_maps


def _combine(results):
    loss = 0.0
    for r in results:
        w_total, q_total = float(r["out"][0, 0]), float(r["out"][0, 1])
        loss += (-w_total / T + PEN * (q_total - JB * K)) / B
    return np.float32(loss)


def run_on_device(preds, y_ts, importance, trace=False, **spmd_kwargs):
    if "nc" not in _CACHE:
        _CACHE["nc"] = _build_program()
    nc = _CACHE["nc"]
    in_maps = _prepare_in_maps(preds, y_ts, importance)
    res = run_bass_kernel_spmd(
        nc, in_maps, list(range(NCORES)), trace=trace, **spmd_kwargs
    )
    return _combine(res.results), res


def kernel(preds, y_ts, importance):
    loss, _ = run_on_device(preds, y_ts, importance, trace=False)
    return loss



# revision 4
# speedup vs baseline: 1.1962x; 1.1962x over previous
"""Trainium2 Bass kernel for AccumulativeGainLoss — fp8-stream version.

Data-parallel over B across 8 NeuronCores (JB=2 batch elements per core).

Math (same restructure as v1, validated on host):
    H    = [F|1]^T [F|1]      bf16 PE, PSUM accum         [33,33]
    inv  = (F^T F)^{-1}       Newton-Schulz 3 iters
    M;sumy = [F|1]^T Y        fp8 DoubleRow PE stream     [34,256]
    sy2  = mask^T Y^2         bf16 PE reduce of squares   row 32
    q    = colsum(M * inv M);  ss_res = sy2 - q
    ss_tot = sy2 - sumy^2/N + EPS;  r2 = 1 - ss_res/ss_tot
    wsum = sum(w * r2);  cov = FtF - s s^T/N; quad = c^T (cov*cov) c
loss = mean_b(-wsum/T) + 0.1 * mean_b(quad - K)

v2 changes vs the 63us bf16 baseline:
- Y ships as fp8e4m3 (3.1MB/core vs 6.2) and the M/sumy stream runs as
  DoubleRow fp8 matmuls (2 k-tiles per matmul, 0.5 cyc/row): PE stream
  drops from ~24k to ~6k cycles.  Host sim: rel err ~7.6e-4 (Y^2 must
  stay bf16; fp8 Y^2 costs 3e-3).
- Y^2 = square(fp8 Y) -> bf16 computed per block, split across
  ScalarE (8 chunks), DVE (4), GpSimd (4); partition-reduced by bf16
  matmuls with the [F|mask] chunk as lhsT (row 32 = sy2).
- Y DMA in 16-chunk blocks ([128 x 4KB] descriptors) chained depth-2.
- PSUM: GS{j} (DR out, also warmup target), Y2S{j}, big{j} (H then
  P then q via tag rotation), tns x2 = exactly 8 banks.
  ss_tot prefix chain on GpSimd so DVE keeps squaring.
"""

import ml_dtypes
import numpy as np

import concourse.bacc as bacc
import concourse.bass as bass
import concourse.mybir as mybir
import concourse.tile as tile
from concourse.bass_utils import run_bass_kernel_spmd
from concourse.tile_rust import add_dep_helper

F32 = mybir.dt.float32
BF16 = mybir.dt.bfloat16
F8 = mybir.dt.float8e4
ALU = mybir.AluOpType
AX = mybir.AxisListType
DR = mybir.MatmulPerfMode.DoubleRow

B, T, N, K, D = 16, 32, 6000, 32, 8
NCORES = 8
JB = B // NCORES          # batch elements per core
NCH = 47                  # ceil(6000/128) real chunks of 128 rows
NCHP = 48                 # padded chunk count (DR pairing)
PAIRS = NCHP // 2         # 24 DoubleRow pair-matmuls per j
TD = T * D                # 256
FW = 34                   # f16 image: 32 coeffs + mask + pad
FROW = NCH * FW           # 1598
KS = 48                   # f8 k-tile stride: dual-fp8 ldweights needs the
                          # outer weight step even and 16B-aligned
PW = 2 * KS               # f8 pair stride
F8ROW = PAIRS * PW        # 2304
YROW = NCHP * TD          # 12288
BLK = 16                  # chunks per Y block (4KB/partition descriptors)
NBLK = NCHP // BLK        # 3 blocks per j
SQ_ACT, SQ_DVE = 8, 4     # chunks squared per block on ScalarE / DVE
NWARM = 10                # PE p-state warmup matmuls
NS_ITERS = 3
EPS = 1e-8
DECAY = 0.9
PEN = 0.1

_CACHE = {}


def _build_program():
    nc = bacc.Bacc("TRN2", target_bir_lowering=False, debug=False)
    y_d = nc.declare_dram_parameter("y", [JB, 128, YROW], F8, isOutput=False)
    f_d = nc.declare_dram_parameter("f", [JB, 128, FROW], BF16, isOutput=False)
    g_d = nc.declare_dram_parameter("g", [JB, 128, F8ROW], F8, isOutput=False)
    c_d = nc.declare_dram_parameter("c32", [32, 96], F32, isOutput=False)
    w_d = nc.declare_dram_parameter("w2", [1, TD], F32, isOutput=False)
    o_d = nc.declare_dram_parameter("out", [1, 2], F32, isOutput=True)

    with tile.TileContext(nc) as tc:
        with (
            tc.tile_pool(name="cpool", bufs=1) as cpool,
            tc.tile_pool(name="fpool", bufs=1) as fpool,
            tc.tile_pool(name="ypool", bufs=1) as ypool,
            tc.tile_pool(name="y2pool", bufs=1) as y2pool,
            tc.tile_pool(name="nsb", bufs=2) as nsb,
            tc.tile_pool(name="esb", bufs=2) as esb,
            tc.tile_pool(name="ps", bufs=1, space="PSUM") as ps,
        ):
            # ---- PSUM banks (8 total): GS{j}, Y2S{j}, big{j}, tns x2
            GS = [ps.tile([34, 512], F32, tag=f"GS{j}", name=f"GS{j}")
                  for j in range(JB)]
            Y2S = [ps.tile([33, 512], F32, tag=f"Y2S{j}", name=f"Y2S{j}")
                   for j in range(JB)]

            # ---- PE warmup into the GS banks (overwritten by the real
            # DoubleRow groups, which re-start the accumulation).
            wtile = cpool.tile([128, 512], BF16)
            nc.gpsimd.memset(wtile, 0.01)
            for i in range(NWARM):
                nc.tensor.matmul(GS[i % 2][0:34, 0:512], wtile[:, 0:34],
                                 wtile, start=True, stop=True)

            # ---- DMAs, chained depth-2, triggers alternating SP/ACT rings:
            # f16 (needed by H first), f8, then the 6 Y blocks.
            ftile = fpool.tile([128, JB * FROW], BF16)
            gtile = fpool.tile([128, JB * F8ROW], F8)
            ytiles = {}
            dmas = []
            dma_engines = [nc.sync, nc.scalar]

            def chain_dma(eng, out, in_):
                dma = eng.dma_start(out=out, in_=in_)
                k = len(dmas)
                if k >= 2:
                    add_dep_helper(dma.ins, dmas[k - 2].ins, sync=True,
                                   reason="depth-2 stream chain")
                dmas.append(dma)
                return dma

            for j in range(JB):
                chain_dma(dma_engines[j % 2],
                          ftile[:, j * FROW:(j + 1) * FROW], f_d[j, :, :])
            for j in range(JB):
                chain_dma(dma_engines[j % 2],
                          gtile[:, j * F8ROW:(j + 1) * F8ROW], g_d[j, :, :])
            for j in range(JB):
                for b in range(NBLK):
                    yt = ypool.tile([128, BLK * TD], F8, tag=f"yb{j}_{b}")
                    chain_dma(dma_engines[len(dmas) % 2], yt[:, :],
                              y_d[j, :, b * BLK * TD:(b + 1) * BLK * TD])
                    ytiles[(j, b)] = yt

            consts = cpool.tile([32, 96], F32)
            nc.gpsimd.dma_start(out=consts, in_=c_d[:, :])
            eye = consts[:, 0:32]
            twoI = consts[:, 32:64]
            ones2d = consts[:, 64:96]
            ones32 = consts[:, 64:65]

            w2sb = cpool.tile([33, TD], F32)
            nc.gpsimd.dma_start(out=w2sb[32:33, :], in_=w_d[:, :])
            sumw = cpool.tile([33, 1], F32)
            nc.vector.reduce_sum(sumw[32:33, :], w2sb[32:33, :], axis=AX.X)

            def fch(j, c):  # chunk-c [F|mask] block [128, 33] bf16
                return ftile[:, j * FROW + c * FW: j * FROW + c * FW + 33]

            # ---- H = [F|mask]^T [F|mask] per j, up front (only needs F).
            Hsb_j = [None, None]
            for j in range(JB):
                Hps = ps.tile([33, 512], F32, tag=f"big{j}")
                for c in range(NCH):
                    nc.tensor.matmul(Hps[0:33, 0:33], fch(j, c), fch(j, c),
                                     start=(c == 0), stop=(c == NCH - 1))
                Hsb = nsb.tile([33, 33], F32, tag="Hsb")
                nc.vector.tensor_copy(Hsb, Hps[0:33, 0:33])
                Hsb_j[j] = Hsb

            # ---- Newton-Schulz + corr-penalty step closures (PE steps are
            # interleaved into the stream so the PE FIFO never head-blocks
            # on their DVE inputs).
            inv_sb = [None, None]
            quad_sb = [None, None]

            def make_steps(j):
                state = {}

                def s_trace():
                    Hsb = Hsb_j[j]
                    A = state["A"] = Hsb[0:32, 0:32]
                    state["s_row"] = Hsb[32:33, 0:32]
                    dm = nsb.tile([32, 32], F32, tag="dm")
                    nc.vector.tensor_mul(dm, A, eye)
                    dg = nsb.tile([32, 1], F32, tag="dg")
                    nc.vector.reduce_sum(dg, dm, axis=AX.X)
                    trp = ps.tile([32, 32], F32, tag="tns", bufs=2)
                    nc.tensor.matmul(trp[:, 0:1], ones2d, dg,
                                     start=True, stop=True)
                    rtr = nsb.tile([32, 1], F32, tag="rtr")
                    nc.vector.reciprocal(rtr, trp[:, 0:1])
                    c0v = nsb.tile([32, 1], F32, tag="c0v")
                    nc.vector.tensor_scalar_mul(c0v, rtr, float(K))
                    X = nsb.tile([32, 32], F32, tag="Xns", bufs=2 * NS_ITERS + 4)
                    nc.vector.tensor_scalar(X, eye, c0v, None, ALU.mult)
                    state["X"] = X
                steps = [s_trace]

                def ns_a():
                    t1 = ps.tile([32, 32], F32, tag="tns", bufs=2)
                    nc.tensor.matmul(t1, state["A"], state["X"],
                                     start=True, stop=True)
                    z = nsb.tile([32, 32], F32, tag="Zns",
                                 bufs=2 * NS_ITERS + 2)
                    nc.vector.tensor_sub(z, twoI, t1)
                    state["z"] = z

                def ns_b():
                    x2 = ps.tile([32, 32], F32, tag="tns", bufs=2)
                    nc.tensor.matmul(x2, state["X"], state["z"],
                                     start=True, stop=True)
                    Xn = nsb.tile([32, 32], F32, tag="Xns",
                                  bufs=2 * NS_ITERS + 4)
                    nc.vector.tensor_copy(Xn, x2)
                    state["X"] = Xn
                for _ in range(NS_ITERS):
                    steps += [ns_a, ns_b]

                def c_outer():
                    inv_sb[j] = state["X"]
                    outp = ps.tile([32, 32], F32, tag="tns", bufs=2)
                    nc.tensor.matmul(outp, state["s_row"], state["s_row"],
                                     start=True, stop=True)
                    covn = nsb.tile([32, 32], F32, tag="covn")
                    nc.vector.tensor_scalar_mul(covn, outp, 1.0 / N)
                    cov = nsb.tile([32, 32], F32, tag="cov")
                    nc.vector.tensor_sub(cov, state["A"], covn)
                    dm2 = nsb.tile([32, 32], F32, tag="dm2")
                    nc.vector.tensor_mul(dm2, cov, eye)
                    dg2 = nsb.tile([32, 1], F32, tag="dg2")
                    nc.vector.reduce_sum(dg2, dm2, axis=AX.X)
                    cv = nsb.tile([32, 1], F32, tag="cv")
                    nc.vector.reciprocal(cv, dg2)
                    A2 = nsb.tile([32, 32], F32, tag="A2")
                    nc.vector.tensor_mul(A2, cov, cov)
                    state["cv"] = cv
                    state["A2"] = A2

                def c_u():
                    ups = ps.tile([32, 32], F32, tag="tns", bufs=2)
                    nc.tensor.matmul(ups[:, 0:1], state["A2"], state["cv"],
                                     start=True, stop=True)
                    usb = nsb.tile([32, 1], F32, tag="usb")
                    nc.vector.tensor_copy(usb, ups[:, 0:1])
                    state["usb"] = usb

                def c_q():
                    qd = ps.tile([33, 32], F32, tag="tns", bufs=2)
                    nc.tensor.matmul(qd[32:33, 0:1], state["usb"], state["cv"],
                                     start=True, stop=True)
                    qsb = nsb.tile([33, 1], F32, tag="qsb")
                    nc.vector.tensor_copy(qsb[32:33, :], qd[32:33, 0:1])
                    quad_sb[j] = qsb
                steps += [c_outer, c_u, c_q]
                return steps

            pending = {0: make_steps(0), 1: make_steps(1)}
            wsout = cpool.tile([33, 4], F32)

            y2tiles = {}

            def emit_squares(j, b):
                yt = ytiles[(j, b)]
                y2t = y2pool.tile([128, BLK * TD], BF16, tag=f"y2_{j}_{b}")
                a = SQ_ACT * TD
                d = a + SQ_DVE * TD
                nc.scalar.square(y2t[:, 0:a], yt[:, 0:a])
                nc.vector.tensor_mul(y2t[:, a:d], yt[:, a:d], yt[:, a:d])
                nc.gpsimd.tensor_mul(y2t[:, d:], yt[:, d:], yt[:, d:])
                y2tiles[(j, b)] = y2t

            def emit_dr(j, b):
                yt = ytiles[(j, b)]
                for p in range(BLK // 2):
                    P = b * (BLK // 2) + p
                    lhsT = gtile[:, j * F8ROW + P * PW: j * F8ROW + (P + 1) * PW
                                 ].rearrange("p (two m) -> p two m", two=2
                                             )[:, :, 0:FW]
                    rhs = yt[:, p * 2 * TD:(p + 1) * 2 * TD
                             ].rearrange("p (two f) -> p two f", two=2)
                    nc.tensor.matmul(GS[j][0:34, 0:256], lhsT, rhs,
                                     start=(P == 0), stop=(P == PAIRS - 1),
                                     perf_mode=DR)

            def emit_y2mm(j, b):
                y2t = y2tiles[(j, b)]
                steps = pending[j]
                for lc in range(BLK):
                    c = b * BLK + lc
                    if c >= NCH:
                        continue
                    nc.tensor.matmul(Y2S[j][0:33, 0:256], fch(j, c),
                                     y2t[:, lc * TD:(lc + 1) * TD],
                                     start=(c == 0), stop=(c == NCH - 1))
                    if c % 3 == 2 and steps:
                        steps.pop(0)()

            def epilogue(j):
                while pending[j]:
                    pending[j].pop(0)()
                Gsb = esb.tile([34, 512], F32, tag="Gsb")
                nc.vector.tensor_copy(Gsb[0:34, 0:256], GS[j][0:34, 0:256])
                nc.vector.tensor_copy(Gsb[32:33, 256:512],
                                      Y2S[j][32:33, 0:256])
                M = Gsb[0:32, 0:TD]
                sumy = Gsb[32:33, 0:TD]
                sy2row = Gsb[32:33, TD:2 * TD]

                Pps = ps.tile([33, 512], F32, tag=f"big{j}")
                nc.tensor.matmul(Pps[0:32, 0:TD], inv_sb[j], M,
                                 start=True, stop=True)
                # ss_tot prefix on GpSimd (DVE is busy squaring)
                sumy2 = esb.tile([33, TD], F32, tag="sumy2")
                nc.gpsimd.tensor_mul(sumy2[32:33, :], sumy, sumy)
                sstot_a = esb.tile([33, TD], F32, tag="sstot_a")
                nc.gpsimd.tensor_scalar(
                    sstot_a[32:33, :], sumy2[32:33, :], -1.0 / N, EPS,
                    ALU.mult, ALU.add)
                sstot = esb.tile([33, TD], F32, tag="sstot")
                nc.gpsimd.tensor_add(sstot[32:33, :], sstot_a[32:33, :],
                                     sy2row)
                rec = esb.tile([33, TD], F32, tag="rec")
                nc.vector.reciprocal(rec[32:33, :], sstot[32:33, :])
                wrec = esb.tile([33, TD], F32, tag="wrec")
                nc.vector.tensor_mul(wrec[32:33, :], rec[32:33, :],
                                     w2sb[32:33, :])
                tA = esb.tile([33, TD], F32, tag="tA")
                accA = esb.tile([33, 1], F32, tag="accA")
                nc.vector.scalar_tensor_tensor(
                    tA[32:33, :], sy2row, 1.0, wrec[32:33, :],
                    ALU.mult, ALU.mult, accum_out=accA[32:33, :])
                W = esb.tile([32, TD], F32, tag="W")
                nc.vector.tensor_mul(W, M, Pps[0:32, 0:TD])
                qps = ps.tile([33, 512], F32, tag=f"big{j}")
                nc.tensor.matmul(qps[32:33, 0:TD], ones32, W,
                                 start=True, stop=True)
                tB = esb.tile([33, TD], F32, tag="tB")
                accB = esb.tile([33, 1], F32, tag="accB")
                nc.vector.scalar_tensor_tensor(
                    tB[32:33, :], qps[32:33, 0:TD], 1.0, wrec[32:33, :],
                    ALU.mult, ALU.mult, accum_out=accB[32:33, :])
                d1 = esb.tile([33, 1], F32, tag="d1")
                nc.vector.tensor_sub(d1[32:33, :], sumw[32:33, :],
                                     accA[32:33, :])
                nc.vector.tensor_add(wsout[32:33, j:j + 1], d1[32:33, :],
                                     accB[32:33, :])
                nc.vector.tensor_copy(wsout[32:33, 2 + j:3 + j],
                                      quad_sb[j][32:33, :])

            # ---- stream: per block emit squares + DR; Y2 matmuls lag one
            # block so their squares are long done when the PE arrives.
            blocks = [(j, b) for j in range(JB) for b in range(NBLK)]
            for k, (j, b) in enumerate(blocks):
                emit_squares(j, b)
                emit_dr(j, b)
                if k > 0:
                    pj, pb = blocks[k - 1]
                    emit_y2mm(pj, pb)
                    if (pj, pb) == (0, NBLK - 1):
                        epilogue(0)
            emit_y2mm(1, NBLK - 1)
            epilogue(1)

            outsb = cpool.tile([33, 2], F32)
            nc.vector.tensor_add(outsb[32:33, 0:1], wsout[32:33, 0:1],
                                 wsout[32:33, 1:2])
            nc.vector.tensor_add(outsb[32:33, 1:2], wsout[32:33, 2:3],
                                 wsout[32:33, 3:4])
            nc.sync.dma_start(out=o_d[:, :], in_=outsb[32:33, :])

    nc.compile()
    return nc


def _prepare_in_maps(preds, y_ts, importance):
    preds = np.ascontiguousarray(preds, dtype=np.float32)
    y_ts = np.ascontiguousarray(y_ts, dtype=np.float32)
    importance = np.ascontiguousarray(importance, dtype=np.float32)

    bf16 = ml_dtypes.bfloat16
    f8 = ml_dtypes.float8_e4m3fn
    NPAD = NCHP * 128     # 6144

    # Y image: yimg[b, p, c*TD + t*D + d] = fp8(y_ts[b, t, c*128+p, d])
    ypad = np.zeros((B, T, NPAD, D), dtype=f8)
    ypad[:, :, :N, :] = y_ts.astype(f8)
    yimg = np.ascontiguousarray(
        ypad.reshape(B, T, NCHP, 128, D).transpose(0, 3, 2, 1, 4)
    ).reshape(B, 128, YROW)

    # F bf16 image: fimg[b, p, c*FW + k]; col 32 = valid-mask
    fpad = np.zeros((B, NCH * 128, FW), dtype=bf16)
    fpad[:, :N, :K] = preds.astype(bf16)
    fpad[:, :N, K] = 1.0
    fimg = np.ascontiguousarray(
        fpad.reshape(B, NCH, 128, FW).transpose(0, 2, 1, 3)
    ).reshape(B, 128, FROW)

    # F fp8 image, 48 chunks, pair-major for DoubleRow lhsT; each 34-col
    # k-tile padded to a 48-byte stride (dual-fp8 ldweights alignment)
    gpad = np.zeros((B, NPAD, FW), dtype=f8)
    gpad[:, :N, :K] = preds.astype(f8)
    gpad[:, :N, K] = 1.0
    gch = gpad.reshape(B, NCHP, 128, FW).transpose(0, 2, 1, 3)  # [B,128,48,34]
    gimg = np.zeros((B, 128, NCHP, KS), dtype=f8)
    gimg[:, :, :, :FW] = gch
    gimg = np.ascontiguousarray(gimg).reshape(B, 128, F8ROW)

    c32 = np.zeros((32, 96), dtype=np.float32)
    c32[:, 0:32] = np.eye(32, dtype=np.float32)
    c32[:, 32:64] = 2.0 * np.eye(32, dtype=np.float32)
    c32[:, 64:96] = 1.0

    decay = DECAY ** np.arange(T, dtype=np.float32)
    w2 = (decay[:, None] * importance[None, :].astype(np.float32)).reshape(1, TD)
    w2 = np.ascontiguousarray(w2, dtype=np.float32)

    in_maps = []
    for i in range(NCORES):
        in_maps.append({
            "y": np.ascontiguousarray(yimg[i * JB:(i + 1) * JB]),
            "f": np.ascontiguousarray(fimg[i * JB:(i + 1) * JB]),
            "g": np.ascontiguousarray(gimg[i * JB:(i + 1) * JB]),
            "c32": c32,
            "w2": w2,
        })
    return in_maps


def _combine(results):
    loss = 0.0
    for r in results:
        w_total, q_total = float(r["out"][0, 0]), float(r["out"][0, 1])
        loss += (-w_total / T + PEN * (q_total - JB * K)) / B
    return np.float32(loss)


def run_on_device(preds, y_ts, importance, trace=False, **spmd_kwargs):
    if "nc" not in _CACHE:
        _CACHE["nc"] = _build_program()
    nc = _CACHE["nc"]
    in_maps = _prepare_in_maps(preds, y_ts, importance)
    res = run_bass_kernel_spmd(
        nc, in_maps, list(range(NCORES)), trace=trace, **spmd_kwargs
    )
    return _combine(res.results), res


def kernel(preds, y_ts, importance):
    loss, _ = run_on_device(preds, y_ts, importance, trace=False)
    return loss


# revision 5
# speedup vs baseline: 1.3061x; 1.0919x over previous
"""Trainium2 Bass kernel for AccumulativeGainLoss — fp8-stream version.

Data-parallel over B across 8 NeuronCores (JB=2 batch elements per core).

Math (same restructure as v1, validated on host):
    H    = [F|1]^T [F|1]      bf16 PE, PSUM accum         [33,33]
    inv  = (F^T F)^{-1}       Newton-Schulz 3 iters
    M;sumy = [F|1]^T Y        fp8 DoubleRow PE stream     [34,256]
    sy2  = mask^T Y^2         bf16 PE reduce of squares   row 32
    q    = colsum(M * inv M);  ss_res = sy2 - q
    ss_tot = sy2 - sumy^2/N + EPS;  r2 = 1 - ss_res/ss_tot
    wsum = sum(w * r2);  cov = FtF - s s^T/N; quad = c^T (cov*cov) c
loss = mean_b(-wsum/T) + 0.1 * mean_b(quad - K)

v2 changes vs the 63us bf16 baseline:
- Y ships as fp8e4m3 (3.1MB/core vs 6.2) and the M/sumy stream runs as
  DoubleRow fp8 matmuls (2 k-tiles per matmul, 0.5 cyc/row): PE stream
  drops from ~24k to ~6k cycles.  Host sim: rel err ~7.6e-4 (Y^2 must
  stay bf16; fp8 Y^2 costs 3e-3).
- Y^2 = square(fp8 Y) -> bf16 computed per block, split across
  ScalarE (8 chunks), DVE (4), GpSimd (4); partition-reduced by bf16
  matmuls with the [F|mask] chunk as lhsT (row 32 = sy2).
- Y DMA in 16-chunk blocks ([128 x 4KB] descriptors) chained depth-2.
- PSUM: GS{j} (DR out, also warmup target), Y2S{j}, big{j} (H then
  P then q via tag rotation), tns x2 = exactly 8 banks.
  ss_tot prefix chain on GpSimd so DVE keeps squaring.
"""

import ml_dtypes
import numpy as np

import concourse.bacc as bacc
import concourse.bass as bass
import concourse.mybir as mybir
import concourse.tile as tile
from concourse.bass_utils import run_bass_kernel_spmd
from concourse.tile_rust import add_dep_helper

F32 = mybir.dt.float32
BF16 = mybir.dt.bfloat16
F8 = mybir.dt.float8e4
ALU = mybir.AluOpType
AX = mybir.AxisListType
DR = mybir.MatmulPerfMode.DoubleRow

B, T, N, K, D = 16, 32, 6000, 32, 8
NCORES = 8
JB = B // NCORES          # batch elements per core
NCH = 47                  # ceil(6000/128) real chunks of 128 rows
NCHP = 48                 # padded chunk count (DR pairing)
PAIRS = NCHP // 2         # 24 DoubleRow pair-matmuls per j
TD = T * D                # 256
FW = 34                   # f16 image: 32 coeffs + mask + pad
FROW = NCH * FW           # 1598
KS = 48                   # f8 k-tile stride: dual-fp8 ldweights needs the
                          # outer weight step even and 16B-aligned
PW = 2 * KS               # f8 pair stride
F8ROW = PAIRS * PW        # 2304
YROW = NCHP * TD          # 12288
BLK = 16                  # chunks per Y block (4KB/partition descriptors)
NBLK = NCHP // BLK        # 3 blocks per j
SQ_ACT, SQ_DVE = 9, 5     # chunks squared per block on ScalarE / DVE (rest Pool)
NWARM = 8                 # PE p-state warmup matmuls
NS_ITERS = 3
EPS = 1e-8
DECAY = 0.9
PEN = 0.1

_CACHE = {}


def _build_program():
    nc = bacc.Bacc("TRN2", target_bir_lowering=False, debug=False)
    y_d = nc.declare_dram_parameter("y", [JB, 128, YROW], F8, isOutput=False)
    f_d = nc.declare_dram_parameter("f", [JB, 128, FROW], BF16, isOutput=False)
    g_d = nc.declare_dram_parameter("g", [JB, 128, F8ROW], F8, isOutput=False)
    c_d = nc.declare_dram_parameter("c32", [32, 96], F32, isOutput=False)
    w_d = nc.declare_dram_parameter("w2", [1, TD], F32, isOutput=False)
    o_d = nc.declare_dram_parameter("out", [1, 2], F32, isOutput=True)

    with tile.TileContext(nc) as tc:
        with (
            tc.tile_pool(name="cpool", bufs=1) as cpool,
            tc.tile_pool(name="fpool", bufs=1) as fpool,
            tc.tile_pool(name="ypool", bufs=1) as ypool,
            tc.tile_pool(name="y2pool", bufs=1) as y2pool,
            tc.tile_pool(name="nsb", bufs=2) as nsb,
            tc.tile_pool(name="esb", bufs=2) as esb,
            tc.tile_pool(name="ps", bufs=1, space="PSUM") as ps,
        ):
            # ---- PSUM banks (8 total): GS{j}, Y2S{j}, big{j}, tns x2
            GS = [ps.tile([34, 512], F32, tag=f"GS{j}", name=f"GS{j}")
                  for j in range(JB)]
            Y2S = [ps.tile([33, 512], F32, tag=f"Y2S{j}", name=f"Y2S{j}")
                   for j in range(JB)]

            # ---- PE warmup into the GS banks (overwritten by the real
            # DoubleRow groups, which re-start the accumulation).
            wtile = cpool.tile([128, 512], BF16)
            nc.gpsimd.memset(wtile, 0.01)
            for i in range(NWARM):
                nc.tensor.matmul(GS[i % 2][0:34, 0:512], wtile[:, 0:34],
                                 wtile, start=True, stop=True)

            # ---- DMAs: one HWDGE ring per batch element (sync ring = j0,
            # scalar ring = j1), each chained depth-2 within itself:
            # f16, f8, then the 3 Y blocks.  Both js stream in parallel so
            # the PE can consume blocks in interleaved-j order from ~4us.
            ftile = fpool.tile([128, JB * FROW], BF16)
            gtile = fpool.tile([128, JB * F8ROW], F8)
            ytiles = {}
            ring = {0: [], 1: []}
            dma_engines = [nc.sync, nc.scalar]

            def chain_dma(j, out, in_):
                dma = dma_engines[j].dma_start(out=out, in_=in_)
                q = ring[j]
                if len(q) >= 2:
                    add_dep_helper(dma.ins, q[-2].ins, sync=True,
                                   reason="depth-2 per-ring chain")
                q.append(dma)
                return dma

            for j in range(JB):
                chain_dma(j, ftile[:, j * FROW:(j + 1) * FROW], f_d[j, :, :])
                chain_dma(j, gtile[:, j * F8ROW:(j + 1) * F8ROW], g_d[j, :, :])
            for b in range(NBLK):
                for j in range(JB):
                    yt = ypool.tile([128, BLK * TD], F8, tag=f"yb{j}_{b}")
                    chain_dma(j, yt[:, :],
                              y_d[j, :, b * BLK * TD:(b + 1) * BLK * TD])
                    ytiles[(j, b)] = yt

            consts = cpool.tile([32, 96], F32)
            nc.gpsimd.dma_start(out=consts, in_=c_d[:, :])
            eye = consts[:, 0:32]
            twoI = consts[:, 32:64]
            ones2d = consts[:, 64:96]
            ones32 = consts[:, 64:65]

            w2sb = cpool.tile([33, TD], F32)
            nc.gpsimd.dma_start(out=w2sb[32:33, :], in_=w_d[:, :])
            sumw = cpool.tile([33, 1], F32)
            nc.vector.reduce_sum(sumw[32:33, :], w2sb[32:33, :], axis=AX.X)

            def fch(j, c):  # chunk-c [F|mask] block [128, 33] bf16
                return ftile[:, j * FROW + c * FW: j * FROW + c * FW + 33]

            # ---- H = [F|mask]^T [F|mask] per j, up front (only needs F).
            Hsb_j = [None, None]
            for j in range(JB):
                Hps = ps.tile([33, 512], F32, tag=f"big{j}")
                for c in range(NCH):
                    nc.tensor.matmul(Hps[0:33, 0:33], fch(j, c), fch(j, c),
                                     start=(c == 0), stop=(c == NCH - 1))
                Hsb = nsb.tile([33, 33], F32, tag="Hsb")
                nc.vector.tensor_copy(Hsb, Hps[0:33, 0:33])
                Hsb_j[j] = Hsb

            # ---- Newton-Schulz + corr-penalty step closures (PE steps are
            # interleaved into the stream so the PE FIFO never head-blocks
            # on their DVE inputs).
            inv_sb = [None, None]
            quad_sb = [None, None]

            def make_steps(j):
                state = {}

                def s_trace():
                    Hsb = Hsb_j[j]
                    A = state["A"] = Hsb[0:32, 0:32]
                    state["s_row"] = Hsb[32:33, 0:32]
                    dm = nsb.tile([32, 32], F32, tag="dm")
                    nc.vector.tensor_mul(dm, A, eye)
                    dg = nsb.tile([32, 1], F32, tag="dg")
                    nc.vector.reduce_sum(dg, dm, axis=AX.X)
                    trp = ps.tile([32, 32], F32, tag="tns", bufs=2)
                    nc.tensor.matmul(trp[:, 0:1], ones2d, dg,
                                     start=True, stop=True)
                    rtr = nsb.tile([32, 1], F32, tag="rtr")
                    nc.vector.reciprocal(rtr, trp[:, 0:1])
                    c0v = nsb.tile([32, 1], F32, tag="c0v")
                    nc.vector.tensor_scalar_mul(c0v, rtr, float(K))
                    X = nsb.tile([32, 32], F32, tag="Xns", bufs=2 * NS_ITERS + 4)
                    nc.vector.tensor_scalar(X, eye, c0v, None, ALU.mult)
                    state["X"] = X
                steps = [s_trace]

                def ns_a():
                    t1 = ps.tile([32, 32], F32, tag="tns", bufs=2)
                    nc.tensor.matmul(t1, state["A"], state["X"],
                                     start=True, stop=True)
                    z = nsb.tile([32, 32], F32, tag="Zns",
                                 bufs=2 * NS_ITERS + 2)
                    nc.vector.tensor_sub(z, twoI, t1)
                    state["z"] = z

                def ns_b():
                    x2 = ps.tile([32, 32], F32, tag="tns", bufs=2)
                    nc.tensor.matmul(x2, state["X"], state["z"],
                                     start=True, stop=True)
                    Xn = nsb.tile([32, 32], F32, tag="Xns",
                                  bufs=2 * NS_ITERS + 4)
                    nc.vector.tensor_copy(Xn, x2)
                    state["X"] = Xn
                for _ in range(NS_ITERS):
                    steps += [ns_a, ns_b]

                def c_outer():
                    inv_sb[j] = state["X"]
                    outp = ps.tile([32, 32], F32, tag="tns", bufs=2)
                    nc.tensor.matmul(outp, state["s_row"], state["s_row"],
                                     start=True, stop=True)
                    covn = nsb.tile([32, 32], F32, tag="covn")
                    nc.vector.tensor_scalar_mul(covn, outp, 1.0 / N)
                    cov = nsb.tile([32, 32], F32, tag="cov")
                    nc.vector.tensor_sub(cov, state["A"], covn)
                    dm2 = nsb.tile([32, 32], F32, tag="dm2")
                    nc.vector.tensor_mul(dm2, cov, eye)
                    dg2 = nsb.tile([32, 1], F32, tag="dg2")
                    nc.vector.reduce_sum(dg2, dm2, axis=AX.X)
                    cv = nsb.tile([32, 1], F32, tag="cv")
                    nc.vector.reciprocal(cv, dg2)
                    A2 = nsb.tile([32, 32], F32, tag="A2")
                    nc.vector.tensor_mul(A2, cov, cov)
                    state["cv"] = cv
                    state["A2"] = A2

                def c_u():
                    ups = ps.tile([32, 32], F32, tag="tns", bufs=2)
                    nc.tensor.matmul(ups[:, 0:1], state["A2"], state["cv"],
                                     start=True, stop=True)
                    usb = nsb.tile([32, 1], F32, tag="usb")
                    nc.vector.tensor_copy(usb, ups[:, 0:1])
                    state["usb"] = usb

                def c_q():
                    qd = ps.tile([33, 32], F32, tag="tns", bufs=2)
                    nc.tensor.matmul(qd[32:33, 0:1], state["usb"], state["cv"],
                                     start=True, stop=True)
                    qsb = nsb.tile([33, 1], F32, tag="qsb")
                    nc.vector.tensor_copy(qsb[32:33, :], qd[32:33, 0:1])
                    quad_sb[j] = qsb
                steps += [c_outer, c_u, c_q]
                return steps

            pending = {0: make_steps(0), 1: make_steps(1)}
            wsout = cpool.tile([33, 4], F32)

            y2tiles = {}

            def emit_squares(j, b):
                yt = ytiles[(j, b)]
                y2t = y2pool.tile([128, BLK * TD], BF16, tag=f"y2_{j}_{b}")
                a = SQ_ACT * TD
                d = a + SQ_DVE * TD
                nc.scalar.square(y2t[:, 0:a], yt[:, 0:a])
                nc.vector.tensor_mul(y2t[:, a:d], yt[:, a:d], yt[:, a:d])
                nc.gpsimd.tensor_mul(y2t[:, d:], yt[:, d:], yt[:, d:])
                y2tiles[(j, b)] = y2t

            def emit_dr(j, b):
                yt = ytiles[(j, b)]
                for p in range(BLK // 2):
                    P = b * (BLK // 2) + p
                    lhsT = gtile[:, j * F8ROW + P * PW: j * F8ROW + (P + 1) * PW
                                 ].rearrange("p (two m) -> p two m", two=2
                                             )[:, :, 0:FW]
                    rhs = yt[:, p * 2 * TD:(p + 1) * 2 * TD
                             ].rearrange("p (two f) -> p two f", two=2)
                    nc.tensor.matmul(GS[j][0:34, 0:256], lhsT, rhs,
                                     start=(P == 0), stop=(P == PAIRS - 1),
                                     perf_mode=DR)

            def emit_y2mm(j, b):
                y2t = y2tiles[(j, b)]
                steps = pending[j]
                for lc in range(BLK):
                    c = b * BLK + lc
                    if c >= NCH:
                        continue
                    nc.tensor.matmul(Y2S[j][0:33, 0:256], fch(j, c),
                                     y2t[:, lc * TD:(lc + 1) * TD],
                                     start=(c == 0), stop=(c == NCH - 1))
                    if c % 3 == 2 and steps:
                        steps.pop(0)()

            def epilogue(j):
                while pending[j]:
                    pending[j].pop(0)()
                Gsb = esb.tile([34, 512], F32, tag="Gsb")
                nc.vector.tensor_copy(Gsb[0:34, 0:256], GS[j][0:34, 0:256])
                nc.vector.tensor_copy(Gsb[32:33, 256:512],
                                      Y2S[j][32:33, 0:256])
                M = Gsb[0:32, 0:TD]
                sumy = Gsb[32:33, 0:TD]
                sy2row = Gsb[32:33, TD:2 * TD]

                Pps = ps.tile([33, 512], F32, tag=f"big{j}")
                nc.tensor.matmul(Pps[0:32, 0:TD], inv_sb[j], M,
                                 start=True, stop=True)
                # ss_tot = sy2 - sumy^2/N + EPS in two fused DVE ops
                sstot_a = esb.tile([33, TD], F32, tag="sstot_a")
                nc.vector.scalar_tensor_tensor(
                    sstot_a[32:33, :], sumy, -1.0 / N, sumy,
                    ALU.mult, ALU.mult)
                sstot = esb.tile([33, TD], F32, tag="sstot")
                nc.vector.scalar_tensor_tensor(
                    sstot[32:33, :], sstot_a[32:33, :], EPS, sy2row,
                    ALU.add, ALU.add)
                rec = esb.tile([33, TD], F32, tag="rec")
                nc.vector.reciprocal(rec[32:33, :], sstot[32:33, :])
                wrec = esb.tile([33, TD], F32, tag="wrec")
                nc.vector.tensor_mul(wrec[32:33, :], rec[32:33, :],
                                     w2sb[32:33, :])
                tA = esb.tile([33, TD], F32, tag="tA")
                accA = esb.tile([33, 1], F32, tag="accA")
                nc.vector.scalar_tensor_tensor(
                    tA[32:33, :], sy2row, 1.0, wrec[32:33, :],
                    ALU.mult, ALU.mult, accum_out=accA[32:33, :])
                W = esb.tile([32, TD], F32, tag="W")
                nc.vector.tensor_mul(W, M, Pps[0:32, 0:TD])
                qps = ps.tile([33, 512], F32, tag=f"big{j}")
                nc.tensor.matmul(qps[32:33, 0:TD], ones32, W,
                                 start=True, stop=True)
                tB = esb.tile([33, TD], F32, tag="tB")
                accB = esb.tile([33, 1], F32, tag="accB")
                nc.vector.scalar_tensor_tensor(
                    tB[32:33, :], qps[32:33, 0:TD], 1.0, wrec[32:33, :],
                    ALU.mult, ALU.mult, accum_out=accB[32:33, :])
                d1 = esb.tile([33, 1], F32, tag="d1")
                nc.vector.tensor_sub(d1[32:33, :], sumw[32:33, :],
                                     accA[32:33, :])
                nc.vector.tensor_add(wsout[32:33, j:j + 1], d1[32:33, :],
                                     accB[32:33, :])
                nc.vector.tensor_copy(wsout[32:33, 2 + j:3 + j],
                                      quad_sb[j][32:33, :])

            # ---- stream, j-interleaved to match the two parallel DMA
            # rings: per block emit squares + DR; Y2 matmuls lag one block
            # so their squares are long done when the PE arrives.
            blocks = [(j, b) for b in range(NBLK) for j in range(JB)]
            for k, (j, b) in enumerate(blocks):
                emit_squares(j, b)
                emit_dr(j, b)
                if k > 0:
                    pj, pb = blocks[k - 1]
                    emit_y2mm(pj, pb)
                    if (pj, pb) == (0, NBLK - 1):
                        epilogue(0)
            emit_y2mm(blocks[-1][0], blocks[-1][1])
            epilogue(1)

            outsb = cpool.tile([33, 2], F32)
            nc.vector.tensor_add(outsb[32:33, 0:1], wsout[32:33, 0:1],
                                 wsout[32:33, 1:2])
            nc.vector.tensor_add(outsb[32:33, 1:2], wsout[32:33, 2:3],
                                 wsout[32:33, 3:4])
            nc.sync.dma_start(out=o_d[:, :], in_=outsb[32:33, :])

    nc.compile()
    return nc


def _prepare_in_maps(preds, y_ts, importance):
    preds = np.ascontiguousarray(preds, dtype=np.float32)
    y_ts = np.ascontiguousarray(y_ts, dtype=np.float32)
    importance = np.ascontiguousarray(importance, dtype=np.float32)

    bf16 = ml_dtypes.bfloat16
    f8 = ml_dtypes.float8_e4m3fn
    NPAD = NCHP * 128     # 6144

    # Y image: yimg[b, p, c*TD + t*D + d] = fp8(y_ts[b, t, c*128+p, d])
    ypad = np.zeros((B, T, NPAD, D), dtype=f8)
    ypad[:, :, :N, :] = y_ts.astype(f8)
    yimg = np.ascontiguousarray(
        ypad.reshape(B, T, NCHP, 128, D).transpose(0, 3, 2, 1, 4)
    ).reshape(B, 128, YROW)

    # F bf16 image: fimg[b, p, c*FW + k]; col 32 = valid-mask
    fpad = np.zeros((B, NCH * 128, FW), dtype=bf16)
    fpad[:, :N, :K] = preds.astype(bf16)
    fpad[:, :N, K] = 1.0
    fimg = np.ascontiguousarray(
        fpad.reshape(B, NCH, 128, FW).transpose(0, 2, 1, 3)
    ).reshape(B, 128, FROW)

    # F fp8 image, 48 chunks, pair-major for DoubleRow lhsT; each 34-col
    # k-tile padded to a 48-byte stride (dual-fp8 ldweights alignment)
    gpad = np.zeros((B, NPAD, FW), dtype=f8)
    gpad[:, :N, :K] = preds.astype(f8)
    gpad[:, :N, K] = 1.0
    gch = gpad.reshape(B, NCHP, 128, FW).transpose(0, 2, 1, 3)  # [B,128,48,34]
    gimg = np.zeros((B, 128, NCHP, KS), dtype=f8)
    gimg[:, :, :, :FW] = gch
    gimg = np.ascontiguousarray(gimg).reshape(B, 128, F8ROW)

    c32 = np.zeros((32, 96), dtype=np.float32)
    c32[:, 0:32] = np.eye(32, dtype=np.float32)
    c32[:, 32:64] = 2.0 * np.eye(32, dtype=np.float32)
    c32[:, 64:96] = 1.0

    decay = DECAY ** np.arange(T, dtype=np.float32)
    w2 = (decay[:, None] * importance[None, :].astype(np.float32)).reshape(1, TD)
    w2 = np.ascontiguousarray(w2, dtype=np.float32)

    in_maps = []
    for i in range(NCORES):
        in_maps.append({
            "y": np.ascontiguousarray(yimg[i * JB:(i + 1) * JB]),
            "f": np.ascontiguousarray(fimg[i * JB:(i + 1) * JB]),
            "g": np.ascontiguousarray(gimg[i * JB:(i + 1) * JB]),
            "c32": c32,
            "w2": w2,
        })
    return in_maps


def _combine(results):
    loss = 0.0
    for r in results:
        w_total, q_total = float(r["out"][0, 0]), float(r["out"][0, 1])
        loss += (-w_total / T + PEN * (q_total - JB * K)) / B
    return np.float32(loss)


def run_on_device(preds, y_ts, importance, trace=False, **spmd_kwargs):
    if "nc" not in _CACHE:
        _CACHE["nc"] = _build_program()
    nc = _CACHE["nc"]
    in_maps = _prepare_in_maps(preds, y_ts, importance)
    res = run_bass_kernel_spmd(
        nc, in_maps, list(range(NCORES)), trace=trace, **spmd_kwargs
    )
    return _combine(res.results), res


def kernel(preds, y_ts, importance):
    loss, _ = run_on_device(preds, y_ts, importance, trace=False)
    return loss
